# revision 9
# baseline (speedup 1.0000x reference)
"""Trainium2 Bass kernel for nn_CrossAttTransformer (S=512 subsets, 3 blocks).

Sharding: data-parallel over the subset axis S across 8 cores (64 subsets/core).
Attention is local to a subset; the only cross-subset op is the gather
queries_to_keys into the flattened (S*Q) query table.  The gather table for
block 0 is a pure function of the inputs and is prepared on host; for blocks
1 and 2 each core contributes its updated LN'd activation rows to an
AllGather, then uses indirect DMA to gather its key rows.

Layout strategy on device: activations for matmuls are kept "transposed"
(T-layout: [feature-dim on partitions, tokens on free]) so every projection
runs with the weight as the stationary operand streaming 512 tokens, while
LN / softmax / residual run in row layout ([tokens, feature]).  PE transposes
bridge the two.  All adaptive-LN bias terms (cond @ sb_w) are algebraically
folded into the downstream projection weights on host (sb @ W terms), which
removes whole elementwise passes on the k-side.
"""

import numpy as np

import concourse.bass as bass
import concourse.tile as tile
from concourse import mybir
from concourse.bass import IndirectOffsetOnAxis
from concourse.bass_utils import run_bass_kernel_spmd
from concourse.masks import make_identity

# problem dims (hardcoded per contract)
S, Q, K, C, H, KD, VD, NB, F, PC = 512, 32, 128, 128, 4, 32, 32, 3, 2, 16
NCORES = 8
HKD = H * KD    # 128
HVD = H * VD    # 128
FC = F * C      # 256
F2 = 2 * FC     # 512
EPS = 1e-5

FP = mybir.dt.float32
AF = mybir.ActivationFunctionType
ALU = mybir.AluOpType


def _ln_np(x):
    x = np.asarray(x, np.float32)
    m = x.mean(-1, keepdims=True, dtype=np.float32)
    v = ((x - m) ** 2).mean(-1, keepdims=True, dtype=np.float32)
    return (x - m) / np.sqrt(v + EPS)


def _split_multiwaits(nc, max_waits=1):
    """walrus (axon compile path) allows only one sync-wait per instruction;
    move excess waits onto same-engine NoOps inserted just before."""
    n = 0
    for bb in nc.main_func.blocks:
        out = []
        for ins in bb.instructions:
            si = ins.sync_info
            waits = list(si.on_wait) if (si and si.on_wait) else []
            if len(waits) > max_waits:
                for w in waits[:-max_waits]:
                    nop = mybir.InstNoOp(name=f"I-mw{n}", ins=[], outs=[])
                    n += 1
                    nop.engine = ins.engine
                    nop.sync_info = type(si)(on_wait=[w], on_update=[])
                    out.append(nop)
                si.on_wait = waits[-max_waits:]
            out.append(ins)
        bb.instructions[:] = out
    return n


# ---------------------------------------------------------------------------
# device program
# ---------------------------------------------------------------------------

def build_program(SL, ncores, split_waits=True):
    """One SPMD program; each core handles SL subsets (RQ=32*SL query rows,
    RK=128*SL key slots)."""
    RQ = SL * Q
    RK = SL * K
    NTQ = RQ // 128          # q-row tiles of 128 tokens  (= chunks of 4 subsets)
    NCH = SL // 4            # k-side chunks of 4 subsets (512 key tokens)
    assert SL % 4 == 0 and NTQ == NCH
    NQC = RQ // 512 if RQ >= 512 else 1   # q-token chunks of <=512 for matmuls
    QCW = min(RQ, 512)                    # q-chunk width
    NRG = ncores * RQ        # global table rows

    nc = bass.Bass(num_devices=ncores)

    # ---- I/O ----
    act0 = nc.dram_tensor("act0", [RQ, C], FP, kind="ExternalInput")
    xn0T = nc.dram_tensor("xn0T", [C, RQ], FP, kind="ExternalInput")
    qscT = nc.dram_tensor("qscT", [C, RQ], FP, kind="ExternalInput")
    cnqT = nc.dram_tensor("cnqT", [C, RQ], FP, kind="ExternalInput")
    cnkT = nc.dram_tensor("cnkT", [C, RK], FP, kind="ExternalInput")
    kln0T = nc.dram_tensor("kln0T", [C, RK], FP, kind="ExternalInput")
    idxT = nc.dram_tensor("idxT", [128, SL], mybir.dt.int32, kind="ExternalInput")
    pl = nc.dram_tensor("pl", [NB, SL, H * Q, K], FP, kind="ExternalInput")

    # per-block params, lhsT layouts
    wscq = nc.dram_tensor("wscq", [NB, C, C], FP, kind="ExternalInput")
    scqb = nc.dram_tensor("scqb", [NB, C], FP, kind="ExternalInput")
    wac = nc.dram_tensor("wac", [NB, C, C], FP, kind="ExternalInput")
    acb = nc.dram_tensor("acb", [NB, C], FP, kind="ExternalInput")
    wscf = nc.dram_tensor("wscf", [NB, C, C], FP, kind="ExternalInput")
    scfb = nc.dram_tensor("scfb", [NB, C], FP, kind="ExternalInput")
    wfc = nc.dram_tensor("wfc", [NB, C, C], FP, kind="ExternalInput")
    fcb = nc.dram_tensor("fcb", [NB, C], FP, kind="ExternalInput")
    wq = nc.dram_tensor("wq", [NB, C, HKD], FP, kind="ExternalInput")
    wq_sb = nc.dram_tensor("wq_sb", [NB, C, HKD], FP, kind="ExternalInput")
    qb = nc.dram_tensor("qb", [NB, HKD], FP, kind="ExternalInput")
    wg = nc.dram_tensor("wg", [NB, C, HVD], FP, kind="ExternalInput")
    wg_sb = nc.dram_tensor("wg_sb", [NB, C, HVD], FP, kind="ExternalInput")
    gb = nc.dram_tensor("gb", [NB, HVD], FP, kind="ExternalInput")
    wsck = nc.dram_tensor("wsck", [NB, C, C], FP, kind="ExternalInput")
    sckb = nc.dram_tensor("sckb", [NB, C], FP, kind="ExternalInput")
    wk = nc.dram_tensor("wk", [NB, C, HKD], FP, kind="ExternalInput")
    wk_sb = nc.dram_tensor("wk_sb", [NB, C, HKD], FP, kind="ExternalInput")
    wv = nc.dram_tensor("wv", [NB, C, HVD], FP, kind="ExternalInput")
    wv_sb = nc.dram_tensor("wv_sb", [NB, C, HVD], FP, kind="ExternalInput")
    wao = nc.dram_tensor("wao", [NB, HVD, C], FP, kind="ExternalInput")
    w1 = nc.dram_tensor("w1", [NB, C, F2], FP, kind="ExternalInput")
    w1_sb = nc.dram_tensor("w1_sb", [NB, C, F2], FP, kind="ExternalInput")
    w2 = nc.dram_tensor("w2", [NB, FC, C], FP, kind="ExternalInput")

    out_ext = nc.dram_tensor("out", [RQ, C], FP, kind="ExternalOutput")

    with tile.TileContext(nc) as tc:
        with (
            tc.tile_pool(name="singles", bufs=1) as singles,
            tc.tile_pool(name="blockres", bufs=1) as blockres,
            tc.tile_pool(name="params", bufs=2) as pp,
            tc.tile_pool(name="work", bufs=3) as work,
            tc.tile_pool(name="attn", bufs=4) as attn,
            tc.tile_pool(name="plp", bufs=4) as plp,
            tc.tile_pool(name="psum", bufs=8, space="PSUM") as psum,
            tc.tile_pool(name="dram", bufs=1, space="DRAM") as dram,
        ):
            # ---------- resident tiles ----------
            ident = singles.tile([128, 128], FP)
            make_identity(nc, ident)
            eps_s = singles.tile([128, 1], FP)
            nc.vector.memset(eps_s, EPS)

            s_cnqT = singles.tile([128, RQ], FP)
            nc.sync.dma_start(out=s_cnqT, in_=cnqT[:, :])
            s_cnkT = singles.tile([128, RK], FP)
            nc.sync.dma_start(out=s_cnkT, in_=cnkT[:, :])
            s_qscT = singles.tile([128, RQ], FP)
            nc.sync.dma_start(out=s_qscT, in_=qscT[:, :])
            s_idx = singles.tile([128, SL], mybir.dt.int32)
            nc.sync.dma_start(out=s_idx, in_=idxT[:, :])
            s_actR = singles.tile([128, NTQ, C], FP)
            nc.sync.dma_start(
                out=s_actR, in_=act0.rearrange("(t p) c -> p t c", p=128)
            )
            s_xn_next = singles.tile([128, NTQ, C], FP)

            # per-block T-layout activation buffers
            s_sig1 = blockres.tile([128, RQ], FP)   # sigmoid(scale) attn / ffw (reused)
            s_gate1 = blockres.tile([128, RQ], FP)  # sigmoid ac / fc gate (reused)
            s_xT = blockres.tile([128, RQ], FP)     # xq^T then xf^T
            s_qT = blockres.tile([128, RQ], FP)
            s_sigT = blockres.tile([128, RQ], FP)

            # DRAM scratch for the LN'd global tables
            lng_in = dram.tile([RQ, C], FP)
            lng = [None, None]
            if ncores > 1:
                lng[0] = dram.tile([NRG, C], FP, name="lng1", addr_space="Shared")
                lng[1] = dram.tile([NRG, C], FP, name="lng2", addr_space="Shared")
            else:
                lng[0] = dram.tile([NRG, C], FP, name="lng1")
                lng[1] = dram.tile([NRG, C], FP, name="lng2")

            def load_param(t, shape):
                s = pp.tile(shape, FP, tag=t.tensor.name if hasattr(t, "tensor") else None)
                nc.sync.dma_start(out=s, in_=t)
                return s

            def cond_matmul(dst, wt, bias_t, rhs_src, func):
                """dst[:, :] (SBUF [128, RQ]) = func(wt.T @ rhs_src + bias)"""
                for chn in range(NQC):
                    sl = slice(chn * QCW, (chn + 1) * QCW)
                    ps = psum.tile([128, QCW], FP, tag="ps")
                    nc.tensor.matmul(ps, wt, rhs_src[:, sl], start=True, stop=True)
                    nc.scalar.activation(dst[:, sl], ps, func, bias=bias_t, scale=1.0)

            def ln_rowtile(src_ap):
                """LayerNorm rows of [128, C] tile -> new SBUF tile."""
                st6 = attn.tile([128, 6], FP, tag="bnst")
                nc.vector.bn_stats(out=st6, in_=src_ap)
                mv = attn.tile([128, 2], FP, tag="bnmv")
                nc.vector.bn_aggr(out=mv, in_=st6)
                sd = attn.tile([128, 1], FP, tag="bnsd")
                nc.scalar.activation(sd, mv[:, 1:2], AF.Sqrt, bias=eps_s, scale=1.0)
                nc.vector.reciprocal(sd, sd)
                xn = work.tile([128, C], FP, tag="xnr")
                nc.vector.tensor_scalar(
                    out=xn, in0=src_ap, scalar1=mv[:, 0:1], scalar2=sd,
                    op0=ALU.subtract, op1=ALU.mult,
                )
                return xn

            def transpose_to(dst_ap, src_ap):
                """dst[128,128] (SBUF) = src[128,128].T via PE."""
                tp = psum.tile([128, 128], FP, tag="ps")
                nc.tensor.transpose(tp, src_ap, ident)
                nc.scalar.activation(dst_ap, tp, AF.Copy, bias=0.0, scale=1.0)

            for b in range(NB):
                # ---------- params for this block ----------
                p_wscq = load_param(wscq[b], [C, C])
                p_scqb = load_param(scqb[b].rearrange('(c o) -> c o', o=1), [C, 1])
                p_wac = load_param(wac[b], [C, C])
                p_acb = load_param(acb[b].rearrange('(c o) -> c o', o=1), [C, 1])
                p_wq = load_param(wq[b], [C, HKD])
                p_wq_sb = load_param(wq_sb[b], [C, HKD])
                p_qb = load_param(qb[b].rearrange('(c o) -> c o', o=1), [HKD, 1])
                p_wg = load_param(wg[b], [C, HVD])
                p_wg_sb = load_param(wg_sb[b], [C, HVD])
                p_gb = load_param(gb[b].rearrange('(c o) -> c o', o=1), [HVD, 1])
                p_wsck = load_param(wsck[b], [C, C])
                p_sckb = load_param(sckb[b].rearrange('(c o) -> c o', o=1), [C, 1])
                p_wk = load_param(wk[b], [C, HKD])
                p_wk_sb = load_param(wk_sb[b], [C, HKD])
                p_wv = load_param(wv[b], [C, HVD])
                p_wv_sb = load_param(wv_sb[b], [C, HVD])
                p_wao = load_param(wao[b], [HVD, C])

                # ---------- P0a: cond sigmoids for the attention part ----------
                cond_matmul(s_sig1, p_wscq, p_scqb, s_cnqT, AF.Sigmoid)   # scale_q
                cond_matmul(s_gate1, p_wac, p_acb, s_qscT, AF.Sigmoid)    # ac gate

                # ---------- Q: q-side projections ----------
                for t in range(NTQ):
                    fsl = slice(t * 128, (t + 1) * 128)
                    if b == 0:
                        xnT = work.tile([128, 128], FP, tag="xnT")
                        nc.sync.dma_start(out=xnT, in_=xn0T[:, fsl])
                    else:
                        xnT = work.tile([128, 128], FP, tag="xnT")
                        transpose_to(xnT, s_xn_next[:, t, :])
                    nc.vector.tensor_mul(s_xT[:, fsl], s_sig1[:, fsl], xnT)

                for chn in range(NQC):
                    sl = slice(chn * QCW, (chn + 1) * QCW)
                    q_ps = psum.tile([128, QCW], FP, tag="ps")
                    nc.tensor.matmul(q_ps, p_wq, s_xT[:, sl], start=True, stop=False)
                    nc.tensor.matmul(q_ps, p_wq_sb, s_cnqT[:, sl], start=False, stop=True)
                    nc.scalar.activation(s_qT[:, sl], q_ps, AF.Identity, bias=p_qb, scale=1.0)
                    g_ps = psum.tile([128, QCW], FP, tag="ps")
                    nc.tensor.matmul(g_ps, p_wg, s_xT[:, sl], start=True, stop=False)
                    nc.tensor.matmul(g_ps, p_wg_sb, s_cnqT[:, sl], start=False, stop=True)
                    nc.scalar.activation(s_sigT[:, sl], g_ps, AF.Sigmoid, bias=p_gb, scale=1.0)

                # ---------- K + A: k-side and attention, per 4-subset chunk ----------
                for ch in range(NCH):
                    kfs = slice(ch * 512, (ch + 1) * 512)
                    klnT = work.tile([128, 512], FP, tag="klnT")
                    if b == 0:
                        nc.sync.dma_start(out=klnT, in_=kln0T[:, kfs])
                    else:
                        for j in range(4):
                            sub = ch * 4 + j
                            kg = work.tile([128, C], FP, tag="kgather")
                            nc.gpsimd.indirect_dma_start(
                                out=kg, out_offset=None,
                                in_=lng[b - 1][:, :],
                                in_offset=IndirectOffsetOnAxis(
                                    ap=s_idx[:, sub:sub + 1], axis=0),
                            )
                            transpose_to(klnT[:, j * 128:(j + 1) * 128], kg)

                    sck_ps = psum.tile([128, 512], FP, tag="ps")
                    nc.tensor.matmul(sck_ps, p_wsck, s_cnkT[:, kfs], start=True, stop=True)
                    sck_s = work.tile([128, 512], FP, tag="scks")
                    nc.scalar.activation(sck_s, sck_ps, AF.Sigmoid, bias=p_sckb, scale=1.0)
                    xkp = work.tile([128, 512], FP, tag="xkp")
                    nc.vector.tensor_mul(xkp, sck_s, klnT)

                    kT_ps = psum.tile([128, 512], FP, tag="ps")
                    nc.tensor.matmul(kT_ps, p_wk, xkp, start=True, stop=False)
                    nc.tensor.matmul(kT_ps, p_wk_sb, s_cnkT[:, kfs], start=False, stop=True)
                    kT_s = work.tile([128, 512], FP, tag="kTs")
                    nc.scalar.activation(kT_s, kT_ps, AF.Copy, bias=0.0, scale=1.0)

                    vR_ps = psum.tile([128, 4, 128], FP, tag="ps")
                    for j in range(4):
                        ks = slice(ch * 512 + j * 128, ch * 512 + (j + 1) * 128)
                        nc.tensor.matmul(vR_ps[:, j, :], xkp[:, j * 128:(j + 1) * 128],
                                         p_wv, start=True, stop=False)
                        nc.tensor.matmul(vR_ps[:, j, :], s_cnkT[:, ks],
                                         p_wv_sb, start=False, stop=True)
                    vR_s = work.tile([128, 4, 128], FP, tag="vRs")
                    nc.vector.tensor_copy(vR_s, vR_ps)

                    wagT = work.tile([128, 128], FP, tag="wagT")
                    for j in range(4):
                        sub = ch * 4 + j
                        lg_ps = psum.tile([128, 128], FP, tag="ps")
                        for h in range(H):
                            hp = slice(32 * h, 32 * (h + 1))
                            nc.tensor.matmul(
                                lg_ps[hp, :],
                                s_qT[hp, sub * 32:(sub + 1) * 32],
                                kT_s[hp, j * 128:(j + 1) * 128],
                                start=True, stop=True, tile_position=(32 * h, 32 * h),
                            )
                        pl_t = plp.tile([128, 128], FP, tag="plt")
                        nc.sync.dma_start(out=pl_t, in_=pl[b, sub])
                        lg_s = attn.tile([128, 128], FP, tag="lgs")
                        nc.vector.tensor_add(lg_s, lg_ps, pl_t)
                        e_s = attn.tile([128, 128], FP, tag="es")
                        esum = attn.tile([128, 1], FP, tag="esum")
                        nc.scalar.activation(e_s, lg_s, AF.Exp, bias=0.0, scale=1.0,
                                             accum_out=esum)
                        nc.vector.reciprocal(esum, esum)
                        at_s = attn.tile([128, 128], FP, tag="ats")
                        nc.vector.tensor_scalar_mul(at_s, e_s, esum)
                        atT_s = attn.tile([128, 128], FP, tag="atTs")
                        transpose_to(atT_s, at_s)
                        waT_ps = psum.tile([128, 32], FP, tag="ps")
                        for h in range(H):
                            hp = slice(32 * h, 32 * (h + 1))
                            nc.tensor.matmul(
                                waT_ps[hp, :],
                                vR_s[:, j, hp],
                                atT_s[:, hp],
                                start=True, stop=True, tile_position=(0, 32 * h),
                            )
                        nc.vector.tensor_mul(wagT[:, j * 32:(j + 1) * 32], waT_ps,
                                             s_sigT[:, sub * 32:(sub + 1) * 32])

                    qfs = slice(ch * 128, (ch + 1) * 128)
                    ao_ps = psum.tile([128, 128], FP, tag="ps")
                    nc.tensor.matmul(ao_ps, p_wao, wagT, start=True, stop=True)
                    aog = work.tile([128, 128], FP, tag="aog")
                    nc.vector.tensor_mul(aog, ao_ps, s_gate1[:, qfs])
                    tp3 = psum.tile([128, 128], FP, tag="ps")
                    nc.tensor.transpose(tp3, aog, ident)
                    nc.vector.tensor_add(s_actR[:, ch, :], s_actR[:, ch, :], tp3)

                # ---------- P0b: cond sigmoids for the ffw part (reuse buffers) ----------
                p_wscf = load_param(wscf[b], [C, C])
                p_scfb = load_param(scfb[b].rearrange('(c o) -> c o', o=1), [C, 1])
                p_wfc = load_param(wfc[b], [C, C])
                p_fcb = load_param(fcb[b].rearrange('(c o) -> c o', o=1), [C, 1])
                p_w1 = load_param(w1[b], [C, F2])
                p_w1_sb = load_param(w1_sb[b], [C, F2])
                p_w2 = load_param(w2[b].rearrange("(g p) c -> p g c", p=128), [128, 2, C])

                cond_matmul(s_sig1, p_wscf, p_scfb, s_cnqT, AF.Sigmoid)   # scale_f
                cond_matmul(s_gate1, p_wfc, p_fcb, s_qscT, AF.Sigmoid)    # fc gate

                # ---------- F: GLU transition ----------
                for t in range(NTQ):
                    fsl = slice(t * 128, (t + 1) * 128)
                    xf = ln_rowtile(s_actR[:, t, :])
                    xfT = work.tile([128, 128], FP, tag="xnT")
                    transpose_to(xfT, xf)
                    nc.vector.tensor_mul(s_xT[:, fsl], s_sig1[:, fsl], xfT)

                for chn in range(NQC):
                    sl = slice(chn * QCW, (chn + 1) * QCW)
                    g_ps = [psum.tile([128, QCW], FP, tag="ps", name=f"g_ps{gi}")
                            for gi in range(4)]
                    for gi in range(4):
                        gsl = slice(gi * 128, (gi + 1) * 128)
                        nc.tensor.matmul(g_ps[gi], p_w1[:, gsl], s_xT[:, sl],
                                         start=True, stop=False)
                        nc.tensor.matmul(g_ps[gi], p_w1_sb[:, gsl], s_cnqT[:, sl],
                                         start=False, stop=True)
                    gl_s = []
                    for gi in range(2):
                        # silu(a)*b as sigmoid(a)*a*b (CoreSim lacks Silu)
                        a_s = work.tile([128, QCW], FP, tag=f"glu{gi}")
                        nc.scalar.activation(a_s, g_ps[gi], AF.Sigmoid, bias=0.0,
                                             scale=1.0)
                        nc.vector.tensor_mul(a_s, a_s, g_ps[gi])
                        nc.vector.tensor_mul(a_s, a_s, g_ps[2 + gi])
                        gl_s.append(a_s)
                    t_ps = psum.tile([128, QCW], FP, tag="ps")
                    nc.tensor.matmul(t_ps, p_w2[:, 0, :], gl_s[0], start=True, stop=False)
                    nc.tensor.matmul(t_ps, p_w2[:, 1, :], gl_s[1], start=False, stop=True)
                    tf = work.tile([128, QCW], FP, tag="tf")
                    nc.vector.tensor_mul(tf, t_ps, s_gate1[:, sl])
                    for tt in range(QCW // 128):
                        t_abs = chn * (QCW // 128) + tt
                        tp4 = psum.tile([128, 128], FP, tag="ps")
                        nc.tensor.transpose(tp4, tf[:, tt * 128:(tt + 1) * 128], ident)
                        nc.vector.tensor_add(s_actR[:, t_abs, :], s_actR[:, t_abs, :], tp4)

                # ---------- G: publish LN'd table for next block ----------
                if b < NB - 1:
                    for t in range(NTQ):
                        xn = ln_rowtile(s_actR[:, t, :])
                        nc.vector.tensor_copy(s_xn_next[:, t, :], xn)
                        nc.sync.dma_start(out=lng_in[t * 128:(t + 1) * 128, :],
                                          in_=s_xn_next[:, t, :])
                    if ncores > 1:
                        nc.gpsimd.collective_compute(
                            "AllGather", ALU.bypass,
                            replica_groups=[list(range(ncores))],
                            ins=[lng_in.opt()], outs=[lng[b].opt()],
                        )
                    else:
                        nc.sync.dma_start(out=lng[b][:, :], in_=lng_in[:, :])

            for t in range(NTQ):
                nc.sync.dma_start(out=out_ext[t * 128:(t + 1) * 128, :],
                                  in_=s_actR[:, t, :])

    if split_waits:
        _split_multiwaits(nc)
    return nc


# ---------------------------------------------------------------------------
# host side: prep, shard, run, gather
# ---------------------------------------------------------------------------

def prep_params(params):
    p = {k: np.asarray(v, np.float32) for k, v in params.items()}
    out = {}
    gq = p['q_ln_g'][:, :, None]                    # [NB, C, 1]
    wsbq = gq * p['q_sb_w']
    out['wscq'] = gq * p['q_sc_w']
    out['scqb'] = p['q_sc_b']
    qw = p['q_w'].reshape(NB, C, HKD) * (KD ** -0.5)
    out['wq'] = qw
    out['qb'] = p['q_b'].reshape(NB, HKD) * (KD ** -0.5)
    out['wq_sb'] = np.einsum('bcd,bde->bce', wsbq, qw)
    out['wg'] = p['gate_w']
    out['wg_sb'] = np.einsum('bcd,bde->bce', wsbq, p['gate_w'])
    out['gb'] = p['gate_b']
    gk = p['k_ln_g'][:, :, None]
    wsbk = gk * p['k_sb_w']
    out['wsck'] = gk * p['k_sc_w']
    out['sckb'] = p['k_sc_b']
    kw = p['k_w'].reshape(NB, C, HKD)
    vw = p['v_w'].reshape(NB, C, HVD)
    out['wk'] = kw
    out['wk_sb'] = np.einsum('bcd,bde->bce', wsbk, kw)
    out['wv'] = vw
    out['wv_sb'] = np.einsum('bcd,bde->bce', wsbk, vw)
    out['wao'] = p['ao_w']
    out['wac'] = p['ac_w']
    out['acb'] = p['ac_b']
    gf = p['f_ln_g'][:, :, None]
    wsbf = gf * p['f_sb_w']
    out['wscf'] = gf * p['f_sc_w']
    out['scfb'] = p['f_sc_b']
    out['w1'] = p['ffw1_w']
    out['w1_sb'] = np.einsum('bcd,bde->bce', wsbf, p['ffw1_w'])
    out['w2'] = p['ffw2_w']
    out['wfc'] = p['fc_w']
    out['fcb'] = p['fc_b']
    return {k: np.ascontiguousarray(v, dtype=np.float32) for k, v in out.items()}


def prep_inputs(queries_act, queries_mask, queries_to_keys, keys_mask,
                queries_single_cond, keys_single_cond, pair_cond, params,
                SL=S // NCORES, ncores=NCORES):
    """Build per-core input maps."""
    qa = np.asarray(queries_act, np.float32)
    qm = np.asarray(queries_mask, np.float32)
    q2k = np.asarray(queries_to_keys, np.int32)
    km = np.asarray(keys_mask, np.float32)
    qsc = np.asarray(queries_single_cond, np.float32)
    ksc = np.asarray(keys_single_cond, np.float32)
    pc_ = np.asarray(pair_cond, np.float32)
    p = {k: np.asarray(v, np.float32) for k, v in params.items()}

    nS = SL * ncores
    RQ = SL * Q
    ln_tab = _ln_np(qa.reshape(nS * Q, C))          # LN of initial table
    xn0 = ln_tab.reshape(nS, Q, C)

    # pair logits (+ mask bias folded in): [NB, S, H, Q, K]
    pa = _ln_np(pc_) * p['pair_ln_g']
    plq = np.einsum('sqkp,pbh->bshqk',
                    pa.reshape(nS, Q, K, PC), p['pair_w'].astype(np.float32),
                    optimize=True).astype(np.float32)
    bias = (1e9 * (qm - 1.0))[:, None, :, None] * (km - 1.0)[:, None, None, :]
    plq = plq + bias[None].astype(np.float32)     # [1,S,1,Q,K] over [NB,S,H,Q,K]

    pw = prep_params(p)

    in_maps = []
    for c in range(ncores):
        ssl = slice(c * SL, (c + 1) * SL)
        m = dict(pw)
        m['act0'] = qa[ssl].reshape(RQ, C)
        m['xn0T'] = np.ascontiguousarray(xn0[ssl].reshape(RQ, C).T)
        m['qscT'] = np.ascontiguousarray(qsc[ssl].reshape(RQ, C).T)
        m['cnqT'] = np.ascontiguousarray(_ln_np(qsc[ssl].reshape(RQ, C)).T)
        m['cnkT'] = np.ascontiguousarray(_ln_np(ksc[ssl].reshape(SL * K, C)).T)
        m['kln0T'] = np.ascontiguousarray(ln_tab[q2k[ssl].reshape(-1)].T)
        m['idxT'] = np.ascontiguousarray(q2k[ssl].T)           # [K=128, SL]
        m['pl'] = np.ascontiguousarray(
            plq[:, ssl].reshape(NB, SL, H * Q, K))
        m = {k: np.ascontiguousarray(v, dtype=(np.int32 if k == 'idxT' else np.float32))
             for k, v in m.items()}
        in_maps.append(m)
    return in_maps


_PROG_CACHE = {}


def get_program(SL=S // NCORES, ncores=NCORES):
    key = (SL, ncores)
    if key not in _PROG_CACHE:
        _PROG_CACHE[key] = build_program(SL, ncores)
    return _PROG_CACHE[key]


def kernel(queries_act, queries_mask, queries_to_keys, keys_mask,
           queries_single_cond, keys_single_cond, pair_cond, params):
    SL = S // NCORES
    in_maps = prep_inputs(queries_act, queries_mask, queries_to_keys, keys_mask,
                          queries_single_cond, keys_single_cond, pair_cond, params,
                          SL=SL, ncores=NCORES)
    nc = get_program(SL, NCORES)
    res = run_bass_kernel_spmd(nc, in_maps, list(range(NCORES)))
    outs = [res.results[c]['out'].reshape(SL, Q, C) for c in range(NCORES)]
    return np.concatenate(outs, axis=0).astype(np.float32)


# ---------------------------------------------------------------------------
# dev-only benchmarking: run the same program via a reusable (non-donating)
# jitted callable and time K queued executions (slope removes fixed overhead).
# ---------------------------------------------------------------------------

def bench(inputs, k1=8, k2=24):
    import time as _time
    import jax
    from jax.sharding import Mesh, PartitionSpec
    from jax.experimental.shard_map import shard_map
    from concourse import bass2jax, mybir as _mb

    SL = S // NCORES
    in_maps = prep_inputs(**inputs, SL=SL, ncores=NCORES)
    nc = get_program(SL, NCORES)
    bass2jax.install_neuronx_cc_hook()

    partition_name = nc.partition_id_tensor.name if nc.partition_id_tensor else None
    in_names, out_names, out_avals, zero_outs = [], [], [], []
    for alloc in nc.m.functions[0].allocations:
        if not isinstance(alloc, _mb.MemoryLocationSet):
            continue
        name = alloc.memorylocations[0].name
        if alloc.kind == "ExternalInput":
            if name != partition_name:
                in_names.append(name)
        elif alloc.kind == "ExternalOutput":
            shape = tuple(alloc.tensor_shape)
            dtype = _mb.dt.np(alloc.dtype)
            out_names.append(name)
            out_avals.append(jax.core.ShapedArray(shape, dtype))
            zero_outs.append(np.zeros(shape, dtype))
    n_params = len(in_names)
    in_names = in_names + out_names
    if partition_name is not None:
        in_names.append(partition_name)

    def _body(*args):
        operands = list(args)
        if partition_name is not None:
            operands.append(bass2jax.partition_id_tensor())
        return tuple(bass2jax._bass_exec_p.bind(
            *operands,
            out_avals=tuple(out_avals),
            in_names=tuple(in_names),
            out_names=tuple(out_names),
            lowering_input_output_aliases=(),
            sim_require_finite=True,
            sim_require_nnan=True,
            nc=nc,
        ))

    devices = jax.devices()[:NCORES]
    mesh = Mesh(np.asarray(devices), ("core",))
    nin = n_params + len(zero_outs)
    jitf = jax.jit(
        shard_map(_body, mesh=mesh,
                  in_specs=(PartitionSpec("core"),) * nin,
                  out_specs=(PartitionSpec("core"),) * len(out_names),
                  check_rep=False),
        keep_unused=True,
    )
    concat_in = [
        np.concatenate([np.asarray(in_maps[c][nm]) for c in range(NCORES)], axis=0)
        for nm in in_names[:n_params]
    ]
    concat_zeros = [
        np.zeros((NCORES * z.shape[0], *z.shape[1:]), z.dtype) for z in zero_outs
    ]
    args = [jax.device_put(a) for a in concat_in + concat_zeros]
    outs = jitf(*args)
    jax.block_until_ready(outs)
    full = np.asarray(outs[0]).reshape(S, Q, C).astype(np.float32)

    def run_k(k):
        t0 = _time.perf_counter()
        rs = [jitf(*args) for _ in range(k)]
        jax.block_until_ready(rs)
        return _time.perf_counter() - t0

    run_k(2)  # warm
    t1 = min(run_k(k1) for _ in range(3))
    t2 = min(run_k(k2) for _ in range(3))
    per_iter_ns = (t2 - t1) / (k2 - k1) * 1e9
    return full, per_iter_ns, (t1, t2)
